# revision 1
# baseline (speedup 1.0000x reference)
"""LSNN layer forward on 8 Trainium2 NeuronCores (data-parallel over batch).

Reference math (per batch row):
    L1    = x_t @ W_syn.T + b_syn
    alpha = sigmoid((L1 + u_t) @ W_Tm.T + b_Tm)
    rho   = sigmoid((L1 + b_t) @ W_Tadp.T + b_Tadp)
    b_new = rho * b_t + (1 - rho) * spk
    thr   = 0.01 + 1.8 * b_new
    u_new = u_t + (L1 - u_t) / alpha
    o_spk = (u_new - thr > 0) as f32

Device formulation (activations transposed, [neuron, batch]):
    1/alpha = 1 + exp(-z1),  rho = 1/(1 + exp(-z2))
    u_new - thr = (L1-u)*exp(-z1) + L1 - 1.8*spk - 1.8*(b-spk)/(1+exp(-z2)) - 0.01
The rho branch uses the Sigmoid activation directly (the Exp-only
variant needs a DVE reciprocal, which measured slower).

Sharding: batch 4096 -> 8 shards of 512; weights replicated; no
cross-core communication.

mm1 modes (first matmul precision/speed):
    f32    - native fp32 (4 cyc/row), exact
    f32r   - TF32-like (1 cyc/row), ~1.5e-4 rel err on L1
    bf16x3 - xh@Wh + xl@Wh + xh@Wl with bf16 hi/lo splits (3 cyc/row),
             ~4e-6 rel err (lo*lo term dropped); halves mm1 weight DMA
The sigmoid-branch matmuls always run f32r: their rounding only moves
values through a heavily damped sigmoid path (measured: zero spike
flips from that path alone). State tensors (u/b/spk) ship as bf16
(exact for this problem's zero-filled states); the spike output ships
as uint8 and is widened to f32 on the host.

Measured on trn2 (8 cores, per-core 512x2048 batch shard):
    f32r   ~212-216us, rel-L2 err 8.8e-3 (322/8.4M spike flips)
    bf16x3 ~319us,     rel-L2 err 1.6e-3 (11 flips)
    f32    ~410us,     exact
The kernel is HBM-bandwidth-bound in f32r mode (65MB/core at
~350GB/s effective; weights dominate: 48MB replicated per core).
"""

import os

import numpy as np
import ml_dtypes

import concourse.bacc as bacc
import concourse.tile as tile
import concourse.mybir as mybir
from concourse.bass_utils import run_bass_kernel_spmd

AF = mybir.ActivationFunctionType
ALU = mybir.AluOpType

B, I, O = 4096, 2048, 2048
NCORES = 8
BC = B // NCORES          # 512 batch rows per core
P = 128                   # partitions
KT = I // P               # 16 k-tiles
OT = O // P               # 16 output neuron tiles
THR_MIN = 0.01

F32 = mybir.dt.float32
F32R = mybir.dt.float32r
BF16 = mybir.dt.bfloat16
U8 = mybir.dt.uint8

MM1_MODE = os.environ.get("MM1_MODE", "f32r")
MM23_DT = F32R


def build_nc():
    mm1_dt = {"f32": F32, "f32r": F32R, "bf16x3": BF16}[MM1_MODE]
    nkt1 = 2 * KT if MM1_MODE == "bf16x3" else KT  # weight k-tiles per o-tile

    nc = bacc.Bacc("TRN2", target_bir_lowering=False, debug=False)

    xh_d = nc.dram_tensor("xh", (P, KT, BC), mm1_dt, kind="ExternalInput").ap()
    xl_d = (nc.dram_tensor("xl", (P, KT, BC), BF16, kind="ExternalInput").ap()
            if MM1_MODE == "bf16x3" else None)
    u_d = nc.dram_tensor("u", (OT, P, BC), BF16, kind="ExternalInput").ap()
    b_d = nc.dram_tensor("b", (OT, P, BC), BF16, kind="ExternalInput").ap()
    spk_d = nc.dram_tensor("spk", (OT, P, BC), BF16, kind="ExternalInput").ap()
    wsyn_d = nc.dram_tensor("wsyn", (P, OT, nkt1, P), mm1_dt, kind="ExternalInput").ap()
    wtm_d = nc.dram_tensor("wtm", (P, OT, KT, P), MM23_DT, kind="ExternalInput").ap()
    wtadp_d = nc.dram_tensor("wtadp", (P, OT, KT, P), MM23_DT, kind="ExternalInput").ap()
    bsyn_d = nc.dram_tensor("bsyn", (P, OT), F32, kind="ExternalInput").ap()
    nbtm_d = nc.dram_tensor("nbtm", (P, OT), F32, kind="ExternalInput").ap()
    btadp_d = nc.dram_tensor("btadp", (P, OT), F32, kind="ExternalInput").ap()
    out_d = nc.dram_tensor("out", (OT, P, BC), U8, kind="ExternalOutput").ap()

    with tile.TileContext(nc) as tc:
        with (
            tc.tile_pool(name="persist", bufs=1) as persist,
            tc.tile_pool(name="wpool", bufs=4) as wpool,
            tc.tile_pool(name="iopool", bufs=6) as iopool,
            tc.tile_pool(name="tmp", bufs=12) as tmp,
            tc.tile_pool(name="outp", bufs=3) as outp,
            tc.tile_pool(name="psum1", bufs=2, space="PSUM") as psum1,
            tc.tile_pool(name="psum2", bufs=6, space="PSUM") as psum2,
        ):
            xsb = persist.tile([P, KT, BC], mm1_dt, tag="xsb")
            if MM1_MODE == "bf16x3":
                xlsb = persist.tile([P, KT, BC], BF16, tag="xlsb")
            l1sb = persist.tile([P, OT, BC], F32, tag="l1sb")
            z1sb = persist.tile([P, OT, BC], MM23_DT, tag="z1sb")
            z2sb = persist.tile([P, OT, BC], MM23_DT, tag="z2sb")
            bsyn = persist.tile([P, OT], F32, tag="bsyn")
            nbtm = persist.tile([P, OT], F32, tag="nbtm")
            btadp = persist.tile([P, OT], F32, tag="btadp")

            # weight tile 0 first, then x per k-tile, so the first
            # matmuls can start as soon as ~0.75MB has landed
            h = nkt1 // 2
            w0 = wpool.tile([P, nkt1, P], mm1_dt, tag="w")
            nc.sync.dma_start(w0[:, :h, :], wsyn_d[:, 0, :h, :])
            for k in range(KT):
                nc.sync.dma_start(xsb[:, k, :], xh_d[:, k, :])
            nc.sync.dma_start(w0[:, h:, :], wsyn_d[:, 0, h:, :])
            if MM1_MODE == "bf16x3":
                for k in range(KT):
                    nc.sync.dma_start(xlsb[:, k, :], xl_d[:, k, :])
            nc.sync.dma_start(bsyn[:], bsyn_d[:])
            nc.sync.dma_start(nbtm[:], nbtm_d[:])
            nc.sync.dma_start(btadp[:], btadp_d[:])

            # ---- phase 1: L1 = W_syn @ x (transposed), Z1 = L1+u, Z2 = L1+b
            for t in range(OT):
                if t == 0:
                    w = w0
                else:
                    w = wpool.tile([P, nkt1, P], mm1_dt, tag="w")
                    nc.sync.dma_start(w[:, :h, :], wsyn_d[:, t, :h, :])
                    nc.sync.dma_start(w[:, h:, :], wsyn_d[:, t, h:, :])
                ps = psum1.tile([P, BC], F32)
                if MM1_MODE == "bf16x3":
                    # w[:, :KT] = Wh, w[:, KT:] = Wl
                    chain = [(w[:, k, :], xsb[:, k, :]) for k in range(KT)]
                    chain += [(w[:, k, :], xlsb[:, k, :]) for k in range(KT)]
                    chain += [(w[:, KT + k, :], xsb[:, k, :]) for k in range(KT)]
                else:
                    chain = [(w[:, k, :], xsb[:, k, :]) for k in range(KT)]
                n = len(chain)
                for i, (lhs, rhs) in enumerate(chain):
                    nc.tensor.matmul(ps[:], lhs, rhs,
                                     start=(i == 0), stop=(i == n - 1))
                nc.scalar.activation(l1sb[:, t, :], ps[:], AF.Identity,
                                     bias=bsyn[:, t:t + 1])
                ut = iopool.tile([P, BC], BF16, tag="io")
                nc.sync.dma_start(ut[:], u_d[t])
                bt = iopool.tile([P, BC], BF16, tag="io")
                nc.sync.dma_start(bt[:], b_d[t])
                nc.vector.tensor_add(z1sb[:, t, :], l1sb[:, t, :], ut[:])
                nc.vector.tensor_add(z2sb[:, t, :], l1sb[:, t, :], bt[:])

            # ---- phase 2: alpha/rho branches + fused pointwise tail
            for t in range(OT):
                wa = wpool.tile([P, KT, P], MM23_DT, tag="w")
                nc.sync.dma_start(wa[:], wtm_d[:, t])
                wr = wpool.tile([P, KT, P], MM23_DT, tag="w")
                nc.sync.dma_start(wr[:], wtadp_d[:, t])
                # matmul-independent pointwise work first, so only the
                # short m/m2/d chain sits behind the matmuls
                ut = iopool.tile([P, BC], BF16, tag="io")
                nc.sync.dma_start(ut[:], u_d[t])
                bt = iopool.tile([P, BC], BF16, tag="io")
                nc.sync.dma_start(bt[:], b_d[t])
                spt = iopool.tile([P, BC], BF16, tag="io")
                nc.sync.dma_start(spt[:], spk_d[t])

                l1t = l1sb[:, t, :]
                # u_new - thr = t1*e + (L1 - 1.8*spk) - 1.8*rho*(b-spk) - 0.01
                sp = tmp.tile([P, BC], F32, tag="t")
                nc.scalar.activation(sp[:], spt[:], AF.Copy, scale=-1.8)
                t1 = tmp.tile([P, BC], F32, tag="t")
                nc.vector.tensor_sub(t1[:], l1t, ut[:])
                t2 = tmp.tile([P, BC], F32, tag="t")
                nc.vector.tensor_sub(t2[:], bt[:], spt[:])
                s = tmp.tile([P, BC], F32, tag="t")
                nc.vector.tensor_add(s[:], l1t, sp[:])

                psa = psum2.tile([P, BC], F32, tag="ps2")
                for k in range(KT):
                    nc.tensor.matmul(psa[:], wa[:, k, :], z1sb[:, k, :],
                                     start=(k == 0), stop=(k == KT - 1))
                psr = psum2.tile([P, BC], F32, tag="ps2")
                for k in range(KT):
                    nc.tensor.matmul(psr[:], wr[:, k, :], z2sb[:, k, :],
                                     start=(k == 0), stop=(k == KT - 1))

                # e = exp(-(z1 + b_Tm)) = 1/alpha - 1; rho = sigmoid(z2 + b_Tadp)
                e = tmp.tile([P, BC], F32, tag="t")
                nc.scalar.activation(e[:], psa[:], AF.Exp,
                                     bias=nbtm[:, t:t + 1], scale=-1.0)
                rho = tmp.tile([P, BC], F32, tag="t")
                nc.scalar.activation(rho[:], psr[:], AF.Sigmoid,
                                     bias=btadp[:, t:t + 1])

                m = tmp.tile([P, BC], F32, tag="t")
                nc.vector.tensor_mul(m[:], t1[:], e[:])
                m2 = tmp.tile([P, BC], F32, tag="t")
                nc.vector.tensor_mul(m2[:], rho[:], t2[:])
                m2s = tmp.tile([P, BC], F32, tag="t")
                nc.scalar.activation(m2s[:], m2[:], AF.Copy, scale=1.8)
                d1 = tmp.tile([P, BC], F32, tag="t")
                nc.vector.tensor_add(d1[:], m[:], s[:])
                d = tmp.tile([P, BC], F32, tag="t")
                nc.vector.tensor_sub(d[:], d1[:], m2s[:])
                o = outp.tile([P, BC], U8, tag="o")
                nc.vector.tensor_scalar(o[:], d[:], THR_MIN, None, ALU.is_gt)
                nc.sync.dma_start(out_d[t], o[:])

    nc.compile()
    return nc


def _pack_weight(w: np.ndarray) -> np.ndarray:
    # [O, I] -> [p, o_tile, k_tile, m] with w[t*128+m, k*128+p] at [p, t, k, m]
    return np.ascontiguousarray(w.reshape(OT, P, KT, P).transpose(3, 0, 2, 1))


def _pack_bias(v: np.ndarray) -> np.ndarray:
    return np.ascontiguousarray(v.reshape(OT, P).T)


def _pack_state(v: np.ndarray) -> np.ndarray:
    return np.ascontiguousarray(
        v.reshape(BC, OT, P).transpose(1, 2, 0).astype(ml_dtypes.bfloat16))


def prepare_in_maps(x_t, u_t, b_t, spk, W_syn, b_syn, W_Tm, b_Tm, W_Tadp, b_Tadp):
    W_syn = np.asarray(W_syn, np.float32)
    if MM1_MODE == "bf16x3":
        wh = W_syn.astype(ml_dtypes.bfloat16)
        wl = (W_syn - wh.astype(np.float32)).astype(ml_dtypes.bfloat16)
        # [p, t, 2*KT, m]: first KT k-tiles = Wh, second KT = Wl
        wsyn = np.ascontiguousarray(
            np.concatenate([_pack_weight(wh), _pack_weight(wl)], axis=2))
    else:
        wsyn = _pack_weight(W_syn)
    wtm = _pack_weight(np.asarray(W_Tm, np.float32))
    wtadp = _pack_weight(np.asarray(W_Tadp, np.float32))
    bsyn = _pack_bias(np.asarray(b_syn, np.float32))
    nbtm = _pack_bias(-np.asarray(b_Tm, np.float32))
    btadp = _pack_bias(np.asarray(b_Tadp, np.float32))

    in_maps = []
    for c in range(NCORES):
        sl = slice(c * BC, (c + 1) * BC)
        xc = np.asarray(x_t[sl], np.float32)
        xp = np.ascontiguousarray(xc.reshape(BC, KT, P).transpose(2, 1, 0))
        m = {
            "u": _pack_state(np.asarray(u_t[sl], np.float32)),
            "b": _pack_state(np.asarray(b_t[sl], np.float32)),
            "spk": _pack_state(np.asarray(spk[sl], np.float32)),
            "wsyn": wsyn, "wtm": wtm, "wtadp": wtadp,
            "bsyn": bsyn, "nbtm": nbtm, "btadp": btadp,
        }
        if MM1_MODE == "bf16x3":
            xph = xp.astype(ml_dtypes.bfloat16)
            xpl = (xp - xph.astype(np.float32)).astype(ml_dtypes.bfloat16)
            m["xh"], m["xl"] = xph, xpl
        else:
            m["xh"] = xp
        in_maps.append(m)
    return in_maps


def unpack_output(results) -> np.ndarray:
    # per-core out: [OT, P, BC] u8 -> [BC, O] f32; concat over cores -> [B, O]
    parts = [r["out"].transpose(2, 0, 1).reshape(BC, O).astype(np.float32)
             for r in results]
    return np.ascontiguousarray(np.concatenate(parts, axis=0))


_NC = None


def get_nc():
    global _NC
    if _NC is None:
        _NC = build_nc()
    return _NC


def run_sharded(in_maps, trace=False, **kw):
    nc = get_nc()
    return run_bass_kernel_spmd(nc, in_maps, list(range(NCORES)), trace=trace, **kw)


def kernel(**inputs) -> np.ndarray:
    in_maps = prepare_in_maps(**inputs)
    res = run_sharded(in_maps)
    return unpack_output(res.results)



# revision 2
# speedup vs baseline: 1.7830x; 1.7830x over previous
"""LSNN layer forward on 8 Trainium2 NeuronCores (data-parallel over batch).

Reference math (per batch row):
    L1    = x_t @ W_syn.T + b_syn
    alpha = sigmoid((L1 + u_t) @ W_Tm.T + b_Tm)
    rho   = sigmoid((L1 + b_t) @ W_Tadp.T + b_Tadp)
    b_new = rho * b_t + (1 - rho) * spk
    thr   = 0.01 + 1.8 * b_new
    u_new = u_t + (L1 - u_t) / alpha
    o_spk = (u_new - thr > 0) as f32

Fast path (u_t = b_t = spk = 0, the shipped input distribution):
    b_new = 0, thr = 0.01, u_new = L1 * (1 + exp(-z1)),
    z1 = L1 @ W_Tm.T + b_Tm.  The rho branch is dead code, and since
    1 + exp(-z1) > 1 > 0, z1 only decides the spike in the sliver
    0 < L1 < 0.01 (~0.44% of elements), so the z1 matmul runs in fp8
    (e4m3, DoubleRow perf mode, 0.5 cyc/row).  Host-side numpy check:
    fp8 z1 quantization costs 161 flips; f32r L1 costs ~322; budget at
    the 2e-2 gate is ~1670.
    Per-core traffic: W_syn 16MB (f32r) + W_Tm 4MB (fp8) + x 4MB +
    out 1MB = 25MB; PE: 131k cyc (mm1 f32r) + 33k cyc (mm2 fp8 DR).

General path (nonzero state): the previous f32r kernel, ~231us.

Sharding: batch 4096 -> 8 shards of 512; weights replicated; no
cross-core communication.  kernel() dispatches on host-side
zero-checks of u_t/b_t/spk, so it stays correct for arbitrary inputs.
"""

import os

import numpy as np
import ml_dtypes

import concourse.bacc as bacc
import concourse.tile as tile
import concourse.mybir as mybir
from concourse.bass_utils import run_bass_kernel_spmd

AF = mybir.ActivationFunctionType
ALU = mybir.AluOpType
PM = mybir.MatmulPerfMode

B, I, O = 4096, 2048, 2048
NCORES = 8
BC = B // NCORES          # 512 batch rows per core
P = 128                   # partitions
KT = I // P               # 16 k-tiles
OT = O // P               # 16 output neuron tiles
KTH = KT // 2             # 8 double-row k-tile pairs
THR_MIN = 0.01

F32 = mybir.dt.float32
F32R = mybir.dt.float32r
BF16 = mybir.dt.bfloat16
FP8 = mybir.dt.float8e4
U8 = mybir.dt.uint8
NP_FP8 = ml_dtypes.float8_e4m3

SX = 32.0                 # L1 -> fp8 scale (|L1| < 5.2, fp8 max 240)
SW = 1024.0               # W_Tm -> fp8 scale (|W| < 0.12)

MM1_MODE = os.environ.get("MM1_MODE", "f32r")
MM2_MODE = os.environ.get("MM2_MODE", "fp8")   # fast path: fp8 | bf16
MM23_DT = F32R


# ---------------------------------------------------------------------------
# fast path: u = b = spk = 0
# ---------------------------------------------------------------------------

def build_nc_fast():
    nc = bacc.Bacc("TRN2", target_bir_lowering=False, debug=False)

    xh_d = nc.dram_tensor("xh", (P, KT, BC), F32R, kind="ExternalInput").ap()
    wsyn_d = nc.dram_tensor("wsyn", (P, OT, KT, P), F32R, kind="ExternalInput").ap()
    if MM2_MODE == "fp8":
        wtm_d = nc.dram_tensor("wtm", (P, OT, KTH, 2, P), FP8,
                               kind="ExternalInput").ap()
    else:
        wtm_d = nc.dram_tensor("wtm", (P, OT, KT, P), BF16,
                               kind="ExternalInput").ap()
    bsyn_d = nc.dram_tensor("bsyn", (P, OT), F32, kind="ExternalInput").ap()
    bsynx_d = nc.dram_tensor("bsynx", (P, OT), F32, kind="ExternalInput").ap()
    nbtm_d = nc.dram_tensor("nbtm", (P, OT), F32, kind="ExternalInput").ap()
    out_d = nc.dram_tensor("out", (OT, P, BC), U8, kind="ExternalOutput").ap()

    l1q_dt = FP8 if MM2_MODE == "fp8" else BF16
    exp_scale = -1.0 / (SX * SW) if MM2_MODE == "fp8" else -1.0

    with tile.TileContext(nc) as tc:
        with (
            tc.tile_pool(name="persist", bufs=1) as persist,
            tc.tile_pool(name="wpool", bufs=4) as wpool,
            tc.tile_pool(name="tmp", bufs=8) as tmp,
            tc.tile_pool(name="outp", bufs=3) as outp,
            tc.tile_pool(name="psum1", bufs=2, space="PSUM") as psum1,
            tc.tile_pool(name="psum2", bufs=6, space="PSUM") as psum2,
        ):
            xsb = persist.tile([P, KT, BC], F32R, tag="xsb")
            l1sb = persist.tile([P, OT, BC], F32, tag="l1sb")
            l1q = persist.tile([P, OT, BC], l1q_dt, tag="l1q")
            if MM2_MODE == "fp8":
                wtm = persist.tile([P, OT, KTH, 2, P], FP8, tag="wtm")
            else:
                wtm = persist.tile([P, OT, KT, P], BF16, tag="wtm")
            bsyn = persist.tile([P, OT], F32, tag="bsyn")
            bsynx = persist.tile([P, OT], F32, tag="bsynx")
            nbtm = persist.tile([P, OT], F32, tag="nbtm")

            # weight tile 0 first, then x per k-tile, so the first
            # matmuls start as soon as ~0.75MB has landed
            h = KT // 2
            w0 = wpool.tile([P, KT, P], F32R, tag="w")
            nc.sync.dma_start(w0[:, :h, :], wsyn_d[:, 0, :h, :])
            for k in range(KT):
                nc.sync.dma_start(xsb[:, k, :], xh_d[:, k, :])
            nc.sync.dma_start(w0[:, h:, :], wsyn_d[:, 0, h:, :])
            nc.sync.dma_start(bsyn[:], bsyn_d[:])
            nc.sync.dma_start(bsynx[:], bsynx_d[:])
            nc.sync.dma_start(nbtm[:], nbtm_d[:])

            # ---- phase 1: L1 = W_syn @ x (transposed), quantized copy for mm2
            for t in range(OT):
                if t == 0:
                    w = w0
                else:
                    w = wpool.tile([P, KT, P], F32R, tag="w")
                    nc.sync.dma_start(w[:, :h, :], wsyn_d[:, t, :h, :])
                    nc.sync.dma_start(w[:, h:, :], wsyn_d[:, t, h:, :])
                ps = psum1.tile([P, BC], F32)
                for k in range(KT):
                    nc.tensor.matmul(ps[:], w[:, k, :], xsb[:, k, :],
                                     start=(k == 0), stop=(k == KT - 1))
                nc.scalar.activation(l1sb[:, t, :], ps[:], AF.Identity,
                                     bias=bsyn[:, t:t + 1])
                nc.scalar.activation(l1q[:, t, :], ps[:], AF.Identity,
                                     bias=bsynx[:, t:t + 1],
                                     scale=(SX if MM2_MODE == "fp8" else 1.0))
                # stream the (small) mm2 weights behind the mm1 weights
                nc.sync.dma_start(wtm[:, t], wtm_d[:, t])

            # ---- phase 2: z1 = W_Tm @ L1 (fp8 double-row), spike tail
            for t in range(OT):
                ps2 = psum2.tile([P, BC], F32, tag="ps2")
                if MM2_MODE == "fp8":
                    for j in range(KTH):
                        nc.tensor.matmul(ps2[:], wtm[:, t, j, :, :],
                                         l1q[:, 2 * j:2 * j + 2, :],
                                         start=(j == 0), stop=(j == KTH - 1),
                                         perf_mode=PM.DoubleRow)
                else:
                    for k in range(KT):
                        nc.tensor.matmul(ps2[:], wtm[:, t, k, :], l1q[:, k, :],
                                         start=(k == 0), stop=(k == KT - 1))
                # e = exp(-(z1 + b_Tm)); spike = L1*(1+e) > 0.01
                e = tmp.tile([P, BC], F32, tag="t")
                nc.scalar.activation(e[:], ps2[:], AF.Exp,
                                     bias=nbtm[:, t:t + 1], scale=exp_scale)
                l1t = l1sb[:, t, :]
                m = tmp.tile([P, BC], F32, tag="t")
                nc.vector.tensor_mul(m[:], l1t, e[:])
                d = tmp.tile([P, BC], F32, tag="t")
                nc.vector.tensor_add(d[:], m[:], l1t)
                o = outp.tile([P, BC], U8, tag="o")
                nc.vector.tensor_scalar(o[:], d[:], THR_MIN, None, ALU.is_gt)
                nc.sync.dma_start(out_d[t], o[:])

    nc.compile()
    return nc


def _pack_weight(w: np.ndarray) -> np.ndarray:
    # [O, I] -> [p, o_tile, k_tile, m] with w[t*128+m, k*128+p] at [p, t, k, m]
    return np.ascontiguousarray(w.reshape(OT, P, KT, P).transpose(3, 0, 2, 1))


def _pack_weight_dr(w8: np.ndarray) -> np.ndarray:
    # fp8 DoubleRow: [O, O] -> [p, t, j, i, m] holding w[t*128+m, (2j+i)*128+p]
    return np.ascontiguousarray(
        w8.reshape(OT, P, KTH, 2, P).transpose(4, 0, 2, 3, 1))


def _pack_bias(v: np.ndarray) -> np.ndarray:
    return np.ascontiguousarray(v.reshape(OT, P).T)


def prepare_in_maps_fast(x_t, W_syn, b_syn, W_Tm, b_Tm):
    wsyn = _pack_weight(np.asarray(W_syn, np.float32))
    if MM2_MODE == "fp8":
        w8 = np.clip(np.asarray(W_Tm, np.float32) * SW, -240.0, 240.0)
        wtm = _pack_weight_dr(w8.astype(NP_FP8))
    else:
        wtm = _pack_weight(np.asarray(W_Tm, np.float32)).astype(
            ml_dtypes.bfloat16)
    b_syn = np.asarray(b_syn, np.float32)
    bsyn = _pack_bias(b_syn)
    bsynx = _pack_bias(b_syn * (SX if MM2_MODE == "fp8" else 1.0))
    nbtm = _pack_bias(-np.asarray(b_Tm, np.float32))

    in_maps = []
    for c in range(NCORES):
        xc = np.asarray(x_t[c * BC:(c + 1) * BC], np.float32)
        xp = np.ascontiguousarray(xc.reshape(BC, KT, P).transpose(2, 1, 0))
        in_maps.append({
            "xh": xp, "wsyn": wsyn, "wtm": wtm,
            "bsyn": bsyn, "bsynx": bsynx, "nbtm": nbtm,
        })
    return in_maps


# ---------------------------------------------------------------------------
# general path (nonzero state): previous kernel, unchanged
# ---------------------------------------------------------------------------

def build_nc():
    mm1_dt = {"f32": F32, "f32r": F32R, "bf16x3": BF16}[MM1_MODE]
    nkt1 = 2 * KT if MM1_MODE == "bf16x3" else KT  # weight k-tiles per o-tile

    nc = bacc.Bacc("TRN2", target_bir_lowering=False, debug=False)

    xh_d = nc.dram_tensor("xh", (P, KT, BC), mm1_dt, kind="ExternalInput").ap()
    xl_d = (nc.dram_tensor("xl", (P, KT, BC), BF16, kind="ExternalInput").ap()
            if MM1_MODE == "bf16x3" else None)
    u_d = nc.dram_tensor("u", (OT, P, BC), BF16, kind="ExternalInput").ap()
    b_d = nc.dram_tensor("b", (OT, P, BC), BF16, kind="ExternalInput").ap()
    spk_d = nc.dram_tensor("spk", (OT, P, BC), BF16, kind="ExternalInput").ap()
    wsyn_d = nc.dram_tensor("wsyn", (P, OT, nkt1, P), mm1_dt, kind="ExternalInput").ap()
    wtm_d = nc.dram_tensor("wtm", (P, OT, KT, P), MM23_DT, kind="ExternalInput").ap()
    wtadp_d = nc.dram_tensor("wtadp", (P, OT, KT, P), MM23_DT, kind="ExternalInput").ap()
    bsyn_d = nc.dram_tensor("bsyn", (P, OT), F32, kind="ExternalInput").ap()
    nbtm_d = nc.dram_tensor("nbtm", (P, OT), F32, kind="ExternalInput").ap()
    btadp_d = nc.dram_tensor("btadp", (P, OT), F32, kind="ExternalInput").ap()
    out_d = nc.dram_tensor("out", (OT, P, BC), U8, kind="ExternalOutput").ap()

    with tile.TileContext(nc) as tc:
        with (
            tc.tile_pool(name="persist", bufs=1) as persist,
            tc.tile_pool(name="wpool", bufs=4) as wpool,
            tc.tile_pool(name="iopool", bufs=6) as iopool,
            tc.tile_pool(name="tmp", bufs=12) as tmp,
            tc.tile_pool(name="outp", bufs=3) as outp,
            tc.tile_pool(name="psum1", bufs=2, space="PSUM") as psum1,
            tc.tile_pool(name="psum2", bufs=6, space="PSUM") as psum2,
        ):
            xsb = persist.tile([P, KT, BC], mm1_dt, tag="xsb")
            if MM1_MODE == "bf16x3":
                xlsb = persist.tile([P, KT, BC], BF16, tag="xlsb")
            l1sb = persist.tile([P, OT, BC], F32, tag="l1sb")
            z1sb = persist.tile([P, OT, BC], MM23_DT, tag="z1sb")
            z2sb = persist.tile([P, OT, BC], MM23_DT, tag="z2sb")
            bsyn = persist.tile([P, OT], F32, tag="bsyn")
            nbtm = persist.tile([P, OT], F32, tag="nbtm")
            btadp = persist.tile([P, OT], F32, tag="btadp")

            # weight tile 0 first, then x per k-tile, so the first
            # matmuls can start as soon as ~0.75MB has landed
            h = nkt1 // 2
            w0 = wpool.tile([P, nkt1, P], mm1_dt, tag="w")
            nc.sync.dma_start(w0[:, :h, :], wsyn_d[:, 0, :h, :])
            for k in range(KT):
                nc.sync.dma_start(xsb[:, k, :], xh_d[:, k, :])
            nc.sync.dma_start(w0[:, h:, :], wsyn_d[:, 0, h:, :])
            if MM1_MODE == "bf16x3":
                for k in range(KT):
                    nc.sync.dma_start(xlsb[:, k, :], xl_d[:, k, :])
            nc.sync.dma_start(bsyn[:], bsyn_d[:])
            nc.sync.dma_start(nbtm[:], nbtm_d[:])
            nc.sync.dma_start(btadp[:], btadp_d[:])

            # ---- phase 1: L1 = W_syn @ x (transposed), Z1 = L1+u, Z2 = L1+b
            for t in range(OT):
                if t == 0:
                    w = w0
                else:
                    w = wpool.tile([P, nkt1, P], mm1_dt, tag="w")
                    nc.sync.dma_start(w[:, :h, :], wsyn_d[:, t, :h, :])
                    nc.sync.dma_start(w[:, h:, :], wsyn_d[:, t, h:, :])
                ps = psum1.tile([P, BC], F32)
                if MM1_MODE == "bf16x3":
                    # w[:, :KT] = Wh, w[:, KT:] = Wl
                    chain = [(w[:, k, :], xsb[:, k, :]) for k in range(KT)]
                    chain += [(w[:, k, :], xlsb[:, k, :]) for k in range(KT)]
                    chain += [(w[:, KT + k, :], xsb[:, k, :]) for k in range(KT)]
                else:
                    chain = [(w[:, k, :], xsb[:, k, :]) for k in range(KT)]
                n = len(chain)
                for i, (lhs, rhs) in enumerate(chain):
                    nc.tensor.matmul(ps[:], lhs, rhs,
                                     start=(i == 0), stop=(i == n - 1))
                nc.scalar.activation(l1sb[:, t, :], ps[:], AF.Identity,
                                     bias=bsyn[:, t:t + 1])
                ut = iopool.tile([P, BC], BF16, tag="io")
                nc.sync.dma_start(ut[:], u_d[t])
                bt = iopool.tile([P, BC], BF16, tag="io")
                nc.sync.dma_start(bt[:], b_d[t])
                nc.vector.tensor_add(z1sb[:, t, :], l1sb[:, t, :], ut[:])
                nc.vector.tensor_add(z2sb[:, t, :], l1sb[:, t, :], bt[:])

            # ---- phase 2: alpha/rho branches + fused pointwise tail
            for t in range(OT):
                wa = wpool.tile([P, KT, P], MM23_DT, tag="w")
                nc.sync.dma_start(wa[:], wtm_d[:, t])
                wr = wpool.tile([P, KT, P], MM23_DT, tag="w")
                nc.sync.dma_start(wr[:], wtadp_d[:, t])
                # matmul-independent pointwise work first, so only the
                # short m/m2/d chain sits behind the matmuls
                ut = iopool.tile([P, BC], BF16, tag="io")
                nc.sync.dma_start(ut[:], u_d[t])
                bt = iopool.tile([P, BC], BF16, tag="io")
                nc.sync.dma_start(bt[:], b_d[t])
                spt = iopool.tile([P, BC], BF16, tag="io")
                nc.sync.dma_start(spt[:], spk_d[t])

                l1t = l1sb[:, t, :]
                # u_new - thr = t1*e + (L1 - 1.8*spk) - 1.8*rho*(b-spk) - 0.01
                sp = tmp.tile([P, BC], F32, tag="t")
                nc.scalar.activation(sp[:], spt[:], AF.Copy, scale=-1.8)
                t1 = tmp.tile([P, BC], F32, tag="t")
                nc.vector.tensor_sub(t1[:], l1t, ut[:])
                t2 = tmp.tile([P, BC], F32, tag="t")
                nc.vector.tensor_sub(t2[:], bt[:], spt[:])
                s = tmp.tile([P, BC], F32, tag="t")
                nc.vector.tensor_add(s[:], l1t, sp[:])

                psa = psum2.tile([P, BC], F32, tag="ps2")
                for k in range(KT):
                    nc.tensor.matmul(psa[:], wa[:, k, :], z1sb[:, k, :],
                                     start=(k == 0), stop=(k == KT - 1))
                psr = psum2.tile([P, BC], F32, tag="ps2")
                for k in range(KT):
                    nc.tensor.matmul(psr[:], wr[:, k, :], z2sb[:, k, :],
                                     start=(k == 0), stop=(k == KT - 1))

                # e = exp(-(z1 + b_Tm)) = 1/alpha - 1; rho = sigmoid(z2 + b_Tadp)
                e = tmp.tile([P, BC], F32, tag="t")
                nc.scalar.activation(e[:], psa[:], AF.Exp,
                                     bias=nbtm[:, t:t + 1], scale=-1.0)
                rho = tmp.tile([P, BC], F32, tag="t")
                nc.scalar.activation(rho[:], psr[:], AF.Sigmoid,
                                     bias=btadp[:, t:t + 1])

                m = tmp.tile([P, BC], F32, tag="t")
                nc.vector.tensor_mul(m[:], t1[:], e[:])
                m2 = tmp.tile([P, BC], F32, tag="t")
                nc.vector.tensor_mul(m2[:], rho[:], t2[:])
                m2s = tmp.tile([P, BC], F32, tag="t")
                nc.scalar.activation(m2s[:], m2[:], AF.Copy, scale=1.8)
                d1 = tmp.tile([P, BC], F32, tag="t")
                nc.vector.tensor_add(d1[:], m[:], s[:])
                d = tmp.tile([P, BC], F32, tag="t")
                nc.vector.tensor_sub(d[:], d1[:], m2s[:])
                o = outp.tile([P, BC], U8, tag="o")
                nc.vector.tensor_scalar(o[:], d[:], THR_MIN, None, ALU.is_gt)
                nc.sync.dma_start(out_d[t], o[:])

    nc.compile()
    return nc


def _pack_state(v: np.ndarray) -> np.ndarray:
    return np.ascontiguousarray(
        v.reshape(BC, OT, P).transpose(1, 2, 0).astype(ml_dtypes.bfloat16))


def prepare_in_maps(x_t, u_t, b_t, spk, W_syn, b_syn, W_Tm, b_Tm, W_Tadp, b_Tadp):
    W_syn = np.asarray(W_syn, np.float32)
    if MM1_MODE == "bf16x3":
        wh = W_syn.astype(ml_dtypes.bfloat16)
        wl = (W_syn - wh.astype(np.float32)).astype(ml_dtypes.bfloat16)
        # [p, t, 2*KT, m]: first KT k-tiles = Wh, second KT = Wl
        wsyn = np.ascontiguousarray(
            np.concatenate([_pack_weight(wh), _pack_weight(wl)], axis=2))
    else:
        wsyn = _pack_weight(W_syn)
    wtm = _pack_weight(np.asarray(W_Tm, np.float32))
    wtadp = _pack_weight(np.asarray(W_Tadp, np.float32))
    bsyn = _pack_bias(np.asarray(b_syn, np.float32))
    nbtm = _pack_bias(-np.asarray(b_Tm, np.float32))
    btadp = _pack_bias(np.asarray(b_Tadp, np.float32))

    in_maps = []
    for c in range(NCORES):
        sl = slice(c * BC, (c + 1) * BC)
        xc = np.asarray(x_t[sl], np.float32)
        xp = np.ascontiguousarray(xc.reshape(BC, KT, P).transpose(2, 1, 0))
        m = {
            "u": _pack_state(np.asarray(u_t[sl], np.float32)),
            "b": _pack_state(np.asarray(b_t[sl], np.float32)),
            "spk": _pack_state(np.asarray(spk[sl], np.float32)),
            "wsyn": wsyn, "wtm": wtm, "wtadp": wtadp,
            "bsyn": bsyn, "nbtm": nbtm, "btadp": btadp,
        }
        if MM1_MODE == "bf16x3":
            xph = xp.astype(ml_dtypes.bfloat16)
            xpl = (xp - xph.astype(np.float32)).astype(ml_dtypes.bfloat16)
            m["xh"], m["xl"] = xph, xpl
        else:
            m["xh"] = xp
        in_maps.append(m)
    return in_maps


def unpack_output(results) -> np.ndarray:
    # per-core out: [OT, P, BC] u8 -> [BC, O] f32; concat over cores -> [B, O]
    parts = [r["out"].transpose(2, 0, 1).reshape(BC, O).astype(np.float32)
             for r in results]
    return np.ascontiguousarray(np.concatenate(parts, axis=0))


# ---------------------------------------------------------------------------
# dispatch
# ---------------------------------------------------------------------------

_NC = {}


def get_nc(fast: bool):
    key = "fast" if fast else "general"
    if key not in _NC:
        _NC[key] = build_nc_fast() if fast else build_nc()
    return _NC[key]


def run_sharded(in_maps, fast=False, trace=False, **kw):
    nc = get_nc(fast)
    return run_bass_kernel_spmd(nc, in_maps, list(range(NCORES)), trace=trace, **kw)


def is_fast_ok(inputs) -> bool:
    return not (np.asarray(inputs["u_t"]).any()
                or np.asarray(inputs["b_t"]).any()
                or np.asarray(inputs["spk"]).any())


def kernel(**inputs) -> np.ndarray:
    if is_fast_ok(inputs):
        in_maps = prepare_in_maps_fast(
            inputs["x_t"], inputs["W_syn"], inputs["b_syn"],
            inputs["W_Tm"], inputs["b_Tm"])
        res = run_sharded(in_maps, fast=True)
    else:
        in_maps = prepare_in_maps(**inputs)
        res = run_sharded(in_maps, fast=False)
    return unpack_output(res.results)


# revision 5
# speedup vs baseline: 2.1251x; 1.1919x over previous
"""LSNN layer forward on 8 Trainium2 NeuronCores (data-parallel over batch).

Reference math (per batch row):
    L1    = x_t @ W_syn.T + b_syn
    alpha = sigmoid((L1 + u_t) @ W_Tm.T + b_Tm)
    rho   = sigmoid((L1 + b_t) @ W_Tadp.T + b_Tadp)
    b_new = rho * b_t + (1 - rho) * spk
    thr   = 0.01 + 1.8 * b_new
    u_new = u_t + (L1 - u_t) / alpha
    o_spk = (u_new - thr > 0) as f32

Fast path (u_t = b_t = spk = 0, the shipped input distribution):
    b_new = 0, thr = 0.01, u_new = L1 * (1 + exp(-z1)),
    z1 = L1 @ W_Tm.T + b_Tm.  The rho branch is dead code, and since
    1 + exp(-z1) > 1 > 0, z1 only decides the spike in the sliver
    0 < L1 < 0.01 (~0.44% of elements), so the z1 matmul runs in fp8
    (e4m3, DoubleRow perf mode, 0.5 cyc/row).  Host-side numpy check:
    fp8 z1 quantization costs 161 flips; f32r L1 costs ~322; budget at
    the 2e-2 gate is ~1670.
    Per-core traffic: W_syn 16MB (f32r) + W_Tm 4MB (fp8) + x 4MB +
    out 1MB = 25MB; PE: 131k cyc (mm1 f32r) + 33k cyc (mm2 fp8 DR).

General path (nonzero state): the previous f32r kernel, ~231us.

Sharding: batch 4096 -> 8 shards of 512; weights replicated; no
cross-core communication.  kernel() dispatches on host-side
zero-checks of u_t/b_t/spk, so it stays correct for arbitrary inputs.
"""

import os

import numpy as np
import ml_dtypes

import concourse.bacc as bacc
import concourse.tile as tile
import concourse.mybir as mybir
from concourse.bass_utils import run_bass_kernel_spmd

AF = mybir.ActivationFunctionType
ALU = mybir.AluOpType
PM = mybir.MatmulPerfMode

B, I, O = 4096, 2048, 2048
NCORES = 8
BC = B // NCORES          # 512 batch rows per core
P = 128                   # partitions
KT = I // P               # 16 k-tiles
OT = O // P               # 16 output neuron tiles
KTH = KT // 2             # 8 double-row k-tile pairs
THR_MIN = 0.01

F32 = mybir.dt.float32
F32R = mybir.dt.float32r
BF16 = mybir.dt.bfloat16
FP8 = mybir.dt.float8e4
U8 = mybir.dt.uint8
NP_FP8 = ml_dtypes.float8_e4m3

SX = 32.0                 # L1 -> fp8 scale (|L1| < 5.2, fp8 max 240)
SW = 1024.0               # W_Tm -> fp8 scale (|W| < 0.12)

MM1_MODE = os.environ.get("MM1_MODE", "f32r")
MM2_MODE = os.environ.get("MM2_MODE", "fp8")   # fast path: fp8 | bf16
MM23_DT = F32R


# ---------------------------------------------------------------------------
# fast path: u = b = spk = 0
# ---------------------------------------------------------------------------

FAST_MM1 = os.environ.get("FAST_MM1", "fp16")   # fp16 | f32r
F16 = mybir.dt.float16
OCH = 4                   # o-tiles per output DMA chunk
XCH = 4                   # x k-tiles per input DMA chunk


def build_nc_fast():
    mm1_dt = F16 if FAST_MM1 == "fp16" else F32R

    nc = bacc.Bacc("TRN2", target_bir_lowering=False, debug=False)

    xh_d = nc.dram_tensor("xh", (P, KT, BC), mm1_dt, kind="ExternalInput").ap()
    wsyn_d = nc.dram_tensor("wsyn", (P, OT, KT, P), mm1_dt, kind="ExternalInput").ap()
    if MM2_MODE == "fp8":
        wtm_d = nc.dram_tensor("wtm", (P, OT, KTH, 2, P), FP8,
                               kind="ExternalInput").ap()
    else:
        wtm_d = nc.dram_tensor("wtm", (P, OT, KT, P), BF16,
                               kind="ExternalInput").ap()
    # bias rows: 0 = b_syn, 1 = SX*b_syn, 2 = -b_Tm, 3 = 0.01 - b_syn
    bias_d = nc.dram_tensor("bias4", (P, 4, OT), F32, kind="ExternalInput").ap()
    out_d = nc.dram_tensor("out", (P, OT, BC), U8, kind="ExternalOutput").ap()

    l1q_dt = FP8 if MM2_MODE == "fp8" else BF16
    exp_scale = -1.0 / (SX * SW) if MM2_MODE == "fp8" else -1.0

    with tile.TileContext(nc) as tc:
        with (
            tc.tile_pool(name="persist", bufs=1) as persist,
            tc.tile_pool(name="wpool", bufs=4) as wpool,
            tc.tile_pool(name="tmp", bufs=8) as tmp,
            tc.tile_pool(name="outp", bufs=2) as outp,
            tc.tile_pool(name="psum1", bufs=2, space="PSUM") as psum1,
            tc.tile_pool(name="psum2", bufs=6, space="PSUM") as psum2,
        ):
            xsb = persist.tile([P, KT, BC], mm1_dt, tag="xsb")
            l1sb = persist.tile([P, OT, BC], F32, tag="l1sb")
            csb = persist.tile([P, OT, BC], F32, tag="csb")
            l1q = persist.tile([P, OT, BC], l1q_dt, tag="l1q")
            if MM2_MODE == "fp8":
                wtm = persist.tile([P, OT, KTH, 2, P], FP8, tag="wtm")
            else:
                wtm = persist.tile([P, OT, KT, P], BF16, tag="wtm")
            bias = persist.tile([P, 4, OT], F32, tag="bias")

            # weight tile 0 first, then x in big chunks, so the first
            # matmuls start early while keeping DMA-issue count low
            h = KT // 2
            w0 = wpool.tile([P, KT, P], mm1_dt, tag="w")
            nc.sync.dma_start(w0[:, :h, :], wsyn_d[:, 0, :h, :])
            for c in range(KT // XCH):
                nc.sync.dma_start(xsb[:, c * XCH:(c + 1) * XCH, :],
                                  xh_d[:, c * XCH:(c + 1) * XCH, :])
                if c == 0:
                    nc.sync.dma_start(w0[:, h:, :], wsyn_d[:, 0, h:, :])
            nc.sync.dma_start(bias[:], bias_d[:])

            # ---- phase 1: L1 = W_syn @ x (transposed), quantized copy for mm2
            for t in range(OT):
                if t == 0:
                    w = w0
                else:
                    w = wpool.tile([P, KT, P], mm1_dt, tag="w")
                    nc.sync.dma_start(w[:], wsyn_d[:, t])
                ps = psum1.tile([P, BC], F32)
                for k in range(KT):
                    nc.tensor.matmul(ps[:], w[:, k, :], xsb[:, k, :],
                                     start=(k == 0), stop=(k == KT - 1))
                nc.scalar.activation(l1sb[:, t, :], ps[:], AF.Identity,
                                     bias=bias[:, 0, t:t + 1])
                nc.scalar.activation(l1q[:, t, :], ps[:], AF.Identity,
                                     bias=bias[:, 1, t:t + 1],
                                     scale=(SX if MM2_MODE == "fp8" else 1.0))
                nc.scalar.activation(csb[:, t, :], ps[:], AF.Identity,
                                     bias=bias[:, 3, t:t + 1], scale=-1.0)
                # stream the (small) mm2 weights behind the mm1 weights
                if t % 4 == 3:
                    nc.sync.dma_start(wtm[:, t - 3:t + 1], wtm_d[:, t - 3:t + 1])

            # ---- phase 2: z1 = W_Tm @ L1 (fp8 double-row), spike tail
            oc = None
            for t in range(OT):
                ps2 = psum2.tile([P, BC], F32, tag="ps2")
                if MM2_MODE == "fp8":
                    for j in range(KTH):
                        nc.tensor.matmul(ps2[:], wtm[:, t, j, :, :],
                                         l1q[:, 2 * j:2 * j + 2, :],
                                         start=(j == 0), stop=(j == KTH - 1),
                                         perf_mode=PM.DoubleRow)
                else:
                    for k in range(KT):
                        nc.tensor.matmul(ps2[:], wtm[:, t, k, :], l1q[:, k, :],
                                         start=(k == 0), stop=(k == KT - 1))
                # e = exp(-(z1 + b_Tm)); spike = L1*e > 0.01 - L1
                e = tmp.tile([P, BC], F32, tag="t")
                nc.scalar.activation(e[:], ps2[:], AF.Exp,
                                     bias=bias[:, 2, t:t + 1], scale=exp_scale)
                m = tmp.tile([P, BC], F32, tag="t")
                nc.gpsimd.tensor_mul(m[:], l1sb[:, t, :], e[:])
                if t % OCH == 0:
                    oc = outp.tile([P, OCH, BC], U8, tag="o")
                nc.vector.tensor_tensor(oc[:, t % OCH, :], m[:], csb[:, t, :],
                                        ALU.is_gt)
                if t % OCH == OCH - 1:
                    nc.sync.dma_start(out_d[:, t - OCH + 1:t + 1, :], oc[:])

    nc.compile()
    return nc


def _pack_weight(w: np.ndarray) -> np.ndarray:
    # [O, I] -> [p, o_tile, k_tile, m] with w[t*128+m, k*128+p] at [p, t, k, m]
    return np.ascontiguousarray(w.reshape(OT, P, KT, P).transpose(3, 0, 2, 1))


def _pack_weight_dr(w8: np.ndarray) -> np.ndarray:
    # fp8 DoubleRow: [O, O] -> [p, t, j, i, m] holding w[t*128+m, (2j+i)*128+p]
    return np.ascontiguousarray(
        w8.reshape(OT, P, KTH, 2, P).transpose(4, 0, 2, 3, 1))


def _pack_bias(v: np.ndarray) -> np.ndarray:
    return np.ascontiguousarray(v.reshape(OT, P).T)


def prepare_in_maps_fast(x_t, W_syn, b_syn, W_Tm, b_Tm):
    mm1_np = np.float16 if FAST_MM1 == "fp16" else np.float32
    wsyn = _pack_weight(np.asarray(W_syn, np.float32)).astype(mm1_np)
    if MM2_MODE == "fp8":
        w8 = np.clip(np.asarray(W_Tm, np.float32) * SW, -240.0, 240.0)
        wtm = _pack_weight_dr(w8.astype(NP_FP8))
    else:
        wtm = _pack_weight(np.asarray(W_Tm, np.float32)).astype(
            ml_dtypes.bfloat16)
    b_syn = np.asarray(b_syn, np.float32)
    bias4 = np.stack([
        _pack_bias(b_syn),
        _pack_bias(b_syn * (SX if MM2_MODE == "fp8" else 1.0)),
        _pack_bias(-np.asarray(b_Tm, np.float32)),
        _pack_bias(THR_MIN - b_syn),
    ], axis=1)  # [P, 4, OT]
    bias4 = np.ascontiguousarray(bias4)

    in_maps = []
    for c in range(NCORES):
        xc = np.asarray(x_t[c * BC:(c + 1) * BC], np.float32)
        xp = np.ascontiguousarray(
            xc.reshape(BC, KT, P).transpose(2, 1, 0)).astype(mm1_np)
        in_maps.append({
            "xh": xp, "wsyn": wsyn, "wtm": wtm, "bias4": bias4,
        })
    return in_maps


def unpack_output_fast(results) -> np.ndarray:
    # per-core out: [P, OT, BC] u8 -> [BC, O] f32; concat over cores -> [B, O]
    parts = [r["out"].transpose(2, 1, 0).reshape(BC, O).astype(np.float32)
             for r in results]
    return np.ascontiguousarray(np.concatenate(parts, axis=0))


# ---------------------------------------------------------------------------
# general path (nonzero state): previous kernel, unchanged
# ---------------------------------------------------------------------------

def build_nc():
    mm1_dt = {"f32": F32, "f32r": F32R, "bf16x3": BF16}[MM1_MODE]
    nkt1 = 2 * KT if MM1_MODE == "bf16x3" else KT  # weight k-tiles per o-tile

    nc = bacc.Bacc("TRN2", target_bir_lowering=False, debug=False)

    xh_d = nc.dram_tensor("xh", (P, KT, BC), mm1_dt, kind="ExternalInput").ap()
    xl_d = (nc.dram_tensor("xl", (P, KT, BC), BF16, kind="ExternalInput").ap()
            if MM1_MODE == "bf16x3" else None)
    u_d = nc.dram_tensor("u", (OT, P, BC), BF16, kind="ExternalInput").ap()
    b_d = nc.dram_tensor("b", (OT, P, BC), BF16, kind="ExternalInput").ap()
    spk_d = nc.dram_tensor("spk", (OT, P, BC), BF16, kind="ExternalInput").ap()
    wsyn_d = nc.dram_tensor("wsyn", (P, OT, nkt1, P), mm1_dt, kind="ExternalInput").ap()
    wtm_d = nc.dram_tensor("wtm", (P, OT, KT, P), MM23_DT, kind="ExternalInput").ap()
    wtadp_d = nc.dram_tensor("wtadp", (P, OT, KT, P), MM23_DT, kind="ExternalInput").ap()
    bsyn_d = nc.dram_tensor("bsyn", (P, OT), F32, kind="ExternalInput").ap()
    nbtm_d = nc.dram_tensor("nbtm", (P, OT), F32, kind="ExternalInput").ap()
    btadp_d = nc.dram_tensor("btadp", (P, OT), F32, kind="ExternalInput").ap()
    out_d = nc.dram_tensor("out", (OT, P, BC), U8, kind="ExternalOutput").ap()

    with tile.TileContext(nc) as tc:
        with (
            tc.tile_pool(name="persist", bufs=1) as persist,
            tc.tile_pool(name="wpool", bufs=4) as wpool,
            tc.tile_pool(name="iopool", bufs=6) as iopool,
            tc.tile_pool(name="tmp", bufs=12) as tmp,
            tc.tile_pool(name="outp", bufs=3) as outp,
            tc.tile_pool(name="psum1", bufs=2, space="PSUM") as psum1,
            tc.tile_pool(name="psum2", bufs=6, space="PSUM") as psum2,
        ):
            xsb = persist.tile([P, KT, BC], mm1_dt, tag="xsb")
            if MM1_MODE == "bf16x3":
                xlsb = persist.tile([P, KT, BC], BF16, tag="xlsb")
            l1sb = persist.tile([P, OT, BC], F32, tag="l1sb")
            z1sb = persist.tile([P, OT, BC], MM23_DT, tag="z1sb")
            z2sb = persist.tile([P, OT, BC], MM23_DT, tag="z2sb")
            bsyn = persist.tile([P, OT], F32, tag="bsyn")
            nbtm = persist.tile([P, OT], F32, tag="nbtm")
            btadp = persist.tile([P, OT], F32, tag="btadp")

            # weight tile 0 first, then x per k-tile, so the first
            # matmuls can start as soon as ~0.75MB has landed
            h = nkt1 // 2
            w0 = wpool.tile([P, nkt1, P], mm1_dt, tag="w")
            nc.sync.dma_start(w0[:, :h, :], wsyn_d[:, 0, :h, :])
            for k in range(KT):
                nc.sync.dma_start(xsb[:, k, :], xh_d[:, k, :])
            nc.sync.dma_start(w0[:, h:, :], wsyn_d[:, 0, h:, :])
            if MM1_MODE == "bf16x3":
                for k in range(KT):
                    nc.sync.dma_start(xlsb[:, k, :], xl_d[:, k, :])
            nc.sync.dma_start(bsyn[:], bsyn_d[:])
            nc.sync.dma_start(nbtm[:], nbtm_d[:])
            nc.sync.dma_start(btadp[:], btadp_d[:])

            # ---- phase 1: L1 = W_syn @ x (transposed), Z1 = L1+u, Z2 = L1+b
            for t in range(OT):
                if t == 0:
                    w = w0
                else:
                    w = wpool.tile([P, nkt1, P], mm1_dt, tag="w")
                    nc.sync.dma_start(w[:, :h, :], wsyn_d[:, t, :h, :])
                    nc.sync.dma_start(w[:, h:, :], wsyn_d[:, t, h:, :])
                ps = psum1.tile([P, BC], F32)
                if MM1_MODE == "bf16x3":
                    # w[:, :KT] = Wh, w[:, KT:] = Wl
                    chain = [(w[:, k, :], xsb[:, k, :]) for k in range(KT)]
                    chain += [(w[:, k, :], xlsb[:, k, :]) for k in range(KT)]
                    chain += [(w[:, KT + k, :], xsb[:, k, :]) for k in range(KT)]
                else:
                    chain = [(w[:, k, :], xsb[:, k, :]) for k in range(KT)]
                n = len(chain)
                for i, (lhs, rhs) in enumerate(chain):
                    nc.tensor.matmul(ps[:], lhs, rhs,
                                     start=(i == 0), stop=(i == n - 1))
                nc.scalar.activation(l1sb[:, t, :], ps[:], AF.Identity,
                                     bias=bsyn[:, t:t + 1])
                ut = iopool.tile([P, BC], BF16, tag="io")
                nc.sync.dma_start(ut[:], u_d[t])
                bt = iopool.tile([P, BC], BF16, tag="io")
                nc.sync.dma_start(bt[:], b_d[t])
                nc.vector.tensor_add(z1sb[:, t, :], l1sb[:, t, :], ut[:])
                nc.vector.tensor_add(z2sb[:, t, :], l1sb[:, t, :], bt[:])

            # ---- phase 2: alpha/rho branches + fused pointwise tail
            for t in range(OT):
                wa = wpool.tile([P, KT, P], MM23_DT, tag="w")
                nc.sync.dma_start(wa[:], wtm_d[:, t])
                wr = wpool.tile([P, KT, P], MM23_DT, tag="w")
                nc.sync.dma_start(wr[:], wtadp_d[:, t])
                # matmul-independent pointwise work first, so only the
                # short m/m2/d chain sits behind the matmuls
                ut = iopool.tile([P, BC], BF16, tag="io")
                nc.sync.dma_start(ut[:], u_d[t])
                bt = iopool.tile([P, BC], BF16, tag="io")
                nc.sync.dma_start(bt[:], b_d[t])
                spt = iopool.tile([P, BC], BF16, tag="io")
                nc.sync.dma_start(spt[:], spk_d[t])

                l1t = l1sb[:, t, :]
                # u_new - thr = t1*e + (L1 - 1.8*spk) - 1.8*rho*(b-spk) - 0.01
                sp = tmp.tile([P, BC], F32, tag="t")
                nc.scalar.activation(sp[:], spt[:], AF.Copy, scale=-1.8)
                t1 = tmp.tile([P, BC], F32, tag="t")
                nc.vector.tensor_sub(t1[:], l1t, ut[:])
                t2 = tmp.tile([P, BC], F32, tag="t")
                nc.vector.tensor_sub(t2[:], bt[:], spt[:])
                s = tmp.tile([P, BC], F32, tag="t")
                nc.vector.tensor_add(s[:], l1t, sp[:])

                psa = psum2.tile([P, BC], F32, tag="ps2")
                for k in range(KT):
                    nc.tensor.matmul(psa[:], wa[:, k, :], z1sb[:, k, :],
                                     start=(k == 0), stop=(k == KT - 1))
                psr = psum2.tile([P, BC], F32, tag="ps2")
                for k in range(KT):
                    nc.tensor.matmul(psr[:], wr[:, k, :], z2sb[:, k, :],
                                     start=(k == 0), stop=(k == KT - 1))

                # e = exp(-(z1 + b_Tm)) = 1/alpha - 1; rho = sigmoid(z2 + b_Tadp)
                e = tmp.tile([P, BC], F32, tag="t")
                nc.scalar.activation(e[:], psa[:], AF.Exp,
                                     bias=nbtm[:, t:t + 1], scale=-1.0)
                rho = tmp.tile([P, BC], F32, tag="t")
                nc.scalar.activation(rho[:], psr[:], AF.Sigmoid,
                                     bias=btadp[:, t:t + 1])

                m = tmp.tile([P, BC], F32, tag="t")
                nc.vector.tensor_mul(m[:], t1[:], e[:])
                m2 = tmp.tile([P, BC], F32, tag="t")
                nc.vector.tensor_mul(m2[:], rho[:], t2[:])
                m2s = tmp.tile([P, BC], F32, tag="t")
                nc.scalar.activation(m2s[:], m2[:], AF.Copy, scale=1.8)
                d1 = tmp.tile([P, BC], F32, tag="t")
                nc.vector.tensor_add(d1[:], m[:], s[:])
                d = tmp.tile([P, BC], F32, tag="t")
                nc.vector.tensor_sub(d[:], d1[:], m2s[:])
                o = outp.tile([P, BC], U8, tag="o")
                nc.vector.tensor_scalar(o[:], d[:], THR_MIN, None, ALU.is_gt)
                nc.sync.dma_start(out_d[t], o[:])

    nc.compile()
    return nc


def _pack_state(v: np.ndarray) -> np.ndarray:
    return np.ascontiguousarray(
        v.reshape(BC, OT, P).transpose(1, 2, 0).astype(ml_dtypes.bfloat16))


def prepare_in_maps(x_t, u_t, b_t, spk, W_syn, b_syn, W_Tm, b_Tm, W_Tadp, b_Tadp):
    W_syn = np.asarray(W_syn, np.float32)
    if MM1_MODE == "bf16x3":
        wh = W_syn.astype(ml_dtypes.bfloat16)
        wl = (W_syn - wh.astype(np.float32)).astype(ml_dtypes.bfloat16)
        # [p, t, 2*KT, m]: first KT k-tiles = Wh, second KT = Wl
        wsyn = np.ascontiguousarray(
            np.concatenate([_pack_weight(wh), _pack_weight(wl)], axis=2))
    else:
        wsyn = _pack_weight(W_syn)
    wtm = _pack_weight(np.asarray(W_Tm, np.float32))
    wtadp = _pack_weight(np.asarray(W_Tadp, np.float32))
    bsyn = _pack_bias(np.asarray(b_syn, np.float32))
    nbtm = _pack_bias(-np.asarray(b_Tm, np.float32))
    btadp = _pack_bias(np.asarray(b_Tadp, np.float32))

    in_maps = []
    for c in range(NCORES):
        sl = slice(c * BC, (c + 1) * BC)
        xc = np.asarray(x_t[sl], np.float32)
        xp = np.ascontiguousarray(xc.reshape(BC, KT, P).transpose(2, 1, 0))
        m = {
            "u": _pack_state(np.asarray(u_t[sl], np.float32)),
            "b": _pack_state(np.asarray(b_t[sl], np.float32)),
            "spk": _pack_state(np.asarray(spk[sl], np.float32)),
            "wsyn": wsyn, "wtm": wtm, "wtadp": wtadp,
            "bsyn": bsyn, "nbtm": nbtm, "btadp": btadp,
        }
        if MM1_MODE == "bf16x3":
            xph = xp.astype(ml_dtypes.bfloat16)
            xpl = (xp - xph.astype(np.float32)).astype(ml_dtypes.bfloat16)
            m["xh"], m["xl"] = xph, xpl
        else:
            m["xh"] = xp
        in_maps.append(m)
    return in_maps


def unpack_output(results) -> np.ndarray:
    # per-core out: [OT, P, BC] u8 -> [BC, O] f32; concat over cores -> [B, O]
    parts = [r["out"].transpose(2, 0, 1).reshape(BC, O).astype(np.float32)
             for r in results]
    return np.ascontiguousarray(np.concatenate(parts, axis=0))


# ---------------------------------------------------------------------------
# dispatch
# ---------------------------------------------------------------------------

_NC = {}


def get_nc(fast: bool):
    key = "fast" if fast else "general"
    if key not in _NC:
        _NC[key] = build_nc_fast() if fast else build_nc()
    return _NC[key]


def run_sharded(in_maps, fast=False, trace=False, **kw):
    nc = get_nc(fast)
    return run_bass_kernel_spmd(nc, in_maps, list(range(NCORES)), trace=trace, **kw)


def is_fast_ok(inputs) -> bool:
    return not (np.asarray(inputs["u_t"]).any()
                or np.asarray(inputs["b_t"]).any()
                or np.asarray(inputs["spk"]).any())


def kernel(**inputs) -> np.ndarray:
    if is_fast_ok(inputs):
        in_maps = prepare_in_maps_fast(
            inputs["x_t"], inputs["W_syn"], inputs["b_syn"],
            inputs["W_Tm"], inputs["b_Tm"])
        res = run_sharded(in_maps, fast=True)
        return unpack_output_fast(res.results)
    in_maps = prepare_in_maps(**inputs)
    res = run_sharded(in_maps, fast=False)
    return unpack_output(res.results)
